# revision 1
# baseline (speedup 1.0000x reference)
"""Multi-head attention (B=2, S=2048, E=1024, H=16) on 8 Trainium2 cores.

Sharding: core c -> (batch b = c//4, head-group g = c%4 of 4 heads).
Each core computes Q/K/V projections for its 4 heads (256 features),
full attention for those heads, and a partial output projection
(256 rows of Wo). Host sums the 4 partials per batch element and adds bo.

On-chip layouts (per core):
  qt/kt: (128 feat-part, pair, 2048 tok)  transposed proj outputs; the
         128 partitions hold two heads (64+64) per pair index.
  v:     (128 tok-part, 16 tok-tiles, 4*65): per head 64 dims plus a
         "ones" column produced by an augmented V projection (extra
         output feature with zero weights and bias 1.0); P @ V_aug then
         also yields the softmax denominator row for free.
  scores are computed transposed (key-pos on partitions, query on free)
  so exp runs on ACT along the free dim and P tiles feed P@V directly as
  the moving operand; no transposes anywhere in the pipeline.

All weight matrices are re-laid-out on the host so every DMA is a
contiguous per-partition run.
"""

import numpy as np

B, S, E, H = 2, 2048, 1024, 16
D = 64
NCORES = 8
FPC = 256  # features (head dims) per core = 4 heads
VW = 4 * 65  # V-projection output width incl. ones columns

# 'f32' (exact, 4 cyc/row), 'f32r' (fp32 storage, rounded fast PE mode), 'bf16'
MODE = "bf16"

_PROGRAMS = {}
LAST_RESULT = None
TRACE = False
TRACE_DIR = None


def _build(mode):
    import concourse.tile as tile
    from concourse import bacc, mybir

    f32 = mybir.dt.float32
    if mode == "bf16":
        DT = mybir.dt.bfloat16
    elif mode == "f32r":
        DT = mybir.dt.float32r
    else:
        DT = f32
    # moving-dim block: matmul fp32 PSUM output caps one bank = 512 floats
    NW = 512
    NNB = S // NW

    nc = bacc.Bacc("TRN2", target_bir_lowering=False, debug=False,
                   num_devices=NCORES)

    xq_ap = nc.dram_tensor("xq", [E, S], DT, kind="ExternalInput").ap()
    xk_ap = nc.dram_tensor("xk", [E, S], DT, kind="ExternalInput").ap()
    xv_ap = nc.dram_tensor("xv", [E, S], DT, kind="ExternalInput").ap()
    wq_ap = nc.dram_tensor("wq", [128, 8, FPC], DT, kind="ExternalInput").ap()
    wk_ap = nc.dram_tensor("wk", [128, 8, FPC], DT, kind="ExternalInput").ap()
    wv_ap = nc.dram_tensor("wv", [128, 8, VW], DT, kind="ExternalInput").ap()
    wo_ap = nc.dram_tensor("wo", [128, 2, E], DT, kind="ExternalInput").ap()
    bqk_ap = nc.dram_tensor("bqk", [128, 4], f32, kind="ExternalInput").ap()
    bv_ap = nc.dram_tensor("bv", [1, VW], DT, kind="ExternalInput").ap()
    ones_ap = nc.dram_tensor("ones", [1, 128], DT, kind="ExternalInput").ap()
    y_ap = nc.dram_tensor("y", [S, E], f32, kind="ExternalOutput").ap()

    Exp = mybir.ActivationFunctionType.Exp

    with tile.TileContext(nc) as tc:
        with tc.tile_pool(name="persist", bufs=1) as persist:
            wq_sb = persist.tile([128, 8, FPC], DT, name="wq_sb")
            wk_sb = persist.tile([128, 8, FPC], DT, name="wk_sb")
            wv_sb = persist.tile([128, 8, VW], DT, name="wv_sb")
            wo_sb = persist.tile([128, 2, E], DT, name="wo_sb")
            bqk_sb = persist.tile([128, 4], f32, name="bqk_sb")
            bv_sb = persist.tile([1, VW], DT, name="bv_sb")
            ones_sb = persist.tile([1, 128], DT, name="ones_sb")
            # weights/constants on the GpSimd DGE queue, x loads spread over
            # the Scalar and Sync queues so streams run in parallel.
            nc.gpsimd.dma_start(wq_sb, wq_ap)
            nc.gpsimd.dma_start(wk_sb, wk_ap)
            nc.gpsimd.dma_start(wv_sb, wv_ap)
            nc.gpsimd.dma_start(bqk_sb, bqk_ap)
            nc.gpsimd.dma_start(bv_sb, bv_ap)
            nc.gpsimd.dma_start(ones_sb, ones_ap)
            ones32 = ones_sb[:, 0:64]

            qt_sb = persist.tile([128, 2, S], DT, name="qt_sb")
            kt_sb = persist.tile([128, 2, S], DT, name="kt_sb")
            v_sb = persist.tile([128, 16, VW], DT, name="v_sb")
            at_sb = persist.tile([128, 2, S], DT, name="at_sb")

            resident = (mode == "bf16")  # x_q/x_k fit SBUF only at 2 bytes

            def emit_v_proj(xv_res, pjv, nway=4):
                # ---- V projection: out = (tok-part, 4*65 feat) ----
                # bias row via K=1 ones-matmul; the augmented columns carry
                # zero weights + bias 1.0 -> ones columns for the denominator
                ngrp = 16 // nway  # token tiles per psum group
                gw = S // nway
                for grp in range(nway):
                    psv = [pjv.tile([128, VW], f32, tag="pjv",
                                    name=f"pjv_{grp}_{i}")
                           for i in range(ngrp)]
                    for i in range(ngrp):
                        nc.tensor.matmul(psv[i], ones_sb, bv_sb,
                                         start=True, stop=False)
                    for kt in range(8):
                        for i in range(ngrp):
                            nc.tensor.matmul(
                                psv[i],
                                xv_res[:, kt,
                                       grp * gw + i * 128:
                                       grp * gw + (i + 1) * 128],
                                wv_sb[:, kt, :],
                                start=False, stop=(kt == 7))
                    for i in range(ngrp):
                        tt = grp * ngrp + i
                        nc.vector.tensor_copy(v_sb[:, tt, :], psv[i])

            def emit_qk_pair(p, qkps, xq_res, xk_res):
                # nb-outer accumulation from resident x: 1 psum slot
                for w_sb, out_sb, x_sb, bcol in (
                    (wk_sb, kt_sb, xk_res, 2), (wq_sb, qt_sb, xq_res, 0),
                ):
                    for nb in range(NNB):
                        pj = qkps.tile([128, NW], f32, tag="misc",
                                       name=f"qk_{p}_{bcol}_{nb}")
                        for kt in range(8):
                            nc.tensor.matmul(
                                pj,
                                w_sb[:, kt, p * 128:(p + 1) * 128],
                                x_sb[:, kt, nb * NW:(nb + 1) * NW],
                                start=(kt == 0), stop=(kt == 7))
                        nc.vector.tensor_scalar_add(
                            out_sb[:, p, nb * NW:(nb + 1) * NW], pj,
                            bqk_sb[:, bcol + p:bcol + p + 1])

            def emit_attention_pair(p, scps, pvps, ptpool, smpool):
                for qb in range(NNB):
                    qsl = slice(qb * NW, (qb + 1) * NW)
                    pvt = [pvps.tile([65, NW], f32, tag="pv",
                                     name=f"pv_{qb}_{p}_{hh}")
                           for hh in range(2)]
                    for kt in range(16):
                        # both heads' transposed scores in one 2-bank tile;
                        # a single exp covers the pair
                        s_ = scps.tile([128, 2 * NW], f32, tag="sc",
                                       name=f"sc_{qb}_{p}_{kt}")
                        for hh in range(2):
                            nc.tensor.matmul(
                                s_[:, NW * hh:NW * hh + NW],
                                kt_sb[64 * hh:64 * hh + 64, p,
                                      kt * 128:(kt + 1) * 128],
                                qt_sb[64 * hh:64 * hh + 64, p, qsl],
                                start=True, stop=True)
                        ptt = ptpool.tile([128, 2 * NW], DT, tag="pt",
                                          name=f"pt_{qb}_{p}_{kt}")
                        nc.scalar.activation(ptt, s_, Exp, scale=0.125)
                        for hh in range(2):
                            h = 2 * p + hh
                            nc.tensor.matmul(
                                pvt[hh],
                                v_sb[:, kt, 65 * h:65 * h + 65],
                                ptt[:, NW * hh:NW * hh + NW],
                                start=(kt == 0), stop=(kt == 15))
                    for hh in range(2):
                        # denom row -> SBUF (ACT), broadcast to 64 partitions
                        # via ones-matmul, then fast approximate reciprocal
                        denr = smpool.tile([1, NW], DT, tag="denr",
                                           name=f"dn_{qb}_{p}_{hh}")
                        nc.vector.tensor_copy(denr, pvt[hh][64:65, :])
                        rb = pvps.tile([64, NW], f32, tag="rb", bufs=1,
                                       name=f"rb_{qb}_{p}_{hh}")
                        nc.tensor.matmul(rb, ones32, denr,
                                         start=True, stop=True)
                        rbs = smpool.tile([64, NW], f32, tag="rbs",
                                          name=f"rbs_{qb}_{p}_{hh}")
                        nc.vector.reciprocal_approx_fast(rbs, rb)
                        nc.vector.tensor_mul(
                            at_sb[64 * hh:64 * hh + 64, p, qsl],
                            pvt[hh][0:64, :], rbs)
                    if p == 1:
                        emit_yproj_block(qb, auxps[0])

            def emit_yproj_block(qb, aux):
                ypool = yproj_pool[0]
                for mt in range(4 * qb, 4 * qb + 4):
                    yo = ypool.tile([128, E], f32, tag="yo", name=f"yo_{mt}")
                    for nb in range(2):
                        yp = aux.tile([128, NW], f32, tag="misc",
                                      name=f"yp_{mt}_{nb}")
                        for p2 in range(2):
                            nc.tensor.matmul(
                                yp,
                                at_sb[:, p2, mt * 128:(mt + 1) * 128],
                                wo_sb[:, p2, nb * NW:(nb + 1) * NW],
                                start=(p2 == 0), stop=(p2 == 1))
                        nc.vector.tensor_copy(yo[:, nb * NW:(nb + 1) * NW],
                                              yp)
                    eng = nc.gpsimd if mt % 2 else nc.sync
                    eng.dma_start(y_ap[mt * 128:(mt + 1) * 128, :], yo)

            yproj_pool = []
            auxps = []
            if resident:
                with tc.tile_pool(name="xres", bufs=1) as xres, \
                     tc.tile_pool(name="qkps", bufs=1, space="PSUM") as qkps:
                    auxps.append(qkps)
                    xq_res = xres.tile([128, 8, S], DT, name="xq_res")
                    xk_res = xres.tile([128, 8, S], DT, name="xk_res")
                    xv_res = xres.tile([128, 8, S], DT, name="xv_res")
                    for kt in range(8):
                        nc.scalar.dma_start(xq_res[:, kt, :],
                                            xq_ap[kt * 128:(kt + 1) * 128, :])
                        nc.sync.dma_start(xk_res[:, kt, :],
                                          xk_ap[kt * 128:(kt + 1) * 128, :])
                        nc.gpsimd.dma_start(xv_res[:, kt, :],
                                            xv_ap[kt * 128:(kt + 1) * 128, :])
                    nc.gpsimd.dma_start(wo_sb, wo_ap)
                    emit_qk_pair(0, qkps, xq_res, xk_res)
                    # quartered V proj (4 psum banks) interleaves with the
                    # pair-0 Q/K projection on the PE
                    with tc.tile_pool(name="pjv", bufs=4,
                                      space="PSUM") as pjv:
                        emit_v_proj(xv_res, pjv, nway=4)
                    with tc.tile_pool(name="pt", bufs=8) as ptpool, \
                         tc.tile_pool(name="sm", bufs=2) as smpool, \
                         tc.tile_pool(name="ysb", bufs=3) as ypool, \
                         tc.tile_pool(name="scps", bufs=2,
                                      space="PSUM") as scps, \
                         tc.tile_pool(name="pvps", bufs=2,
                                      space="PSUM") as pvps:
                        yproj_pool.append(ypool)
                        emit_attention_pair(0, scps, pvps, ptpool, smpool)
                        emit_qk_pair(1, qkps, xq_res, xk_res)
                        emit_attention_pair(1, scps, pvps, ptpool, smpool)
            else:
                # fp32 variants: stream x, kt-inner projections (8-bank psum)
                nc.gpsimd.dma_start(wo_sb, wo_ap)
                with tc.tile_pool(name="xs", bufs=3) as xpool:
                    with tc.tile_pool(name="pjqk", bufs=2 * NNB,
                                      space="PSUM") as pjqk:
                        for which, xap, w_sb, out_sb, bcol in (
                            (0, xq_ap, wq_sb, qt_sb, 0),
                            (1, xk_ap, wk_sb, kt_sb, 2),
                        ):
                            ps = {}
                            for mt in range(2):
                                for nb in range(NNB):
                                    ps[(mt, nb)] = pjqk.tile(
                                        [128, NW], f32, tag="pj",
                                        name=f"pjq_{which}_{mt}_{nb}")
                            for kt in range(8):
                                xt = xpool.tile([128, S], DT, tag="x",
                                                name=f"x_{which}_{kt}")
                                nc.sync.dma_start(
                                    xt, xap[kt * 128:(kt + 1) * 128, :])
                                for mt in range(2):
                                    for nb in range(NNB):
                                        nc.tensor.matmul(
                                            ps[(mt, nb)],
                                            w_sb[:, kt,
                                                 mt * 128:(mt + 1) * 128],
                                            xt[:, nb * NW:(nb + 1) * NW],
                                            start=(kt == 0), stop=(kt == 7))
                            for mt in range(2):
                                for nb in range(NNB):
                                    nc.vector.tensor_scalar_add(
                                        out_sb[:, mt, nb * NW:(nb + 1) * NW],
                                        ps[(mt, nb)],
                                        bqk_sb[:, bcol + mt:bcol + mt + 1])
                    with tc.tile_pool(name="pjv", bufs=8,
                                      space="PSUM") as pjv:
                        # streamed V: halves of (128,1024) keep DMA rows 4KB
                        for half in range(2):
                            psv = [pjv.tile([128, VW], f32, tag="pjv",
                                            name=f"pjv_{half}_{i}")
                                   for i in range(8)]
                            for i in range(8):
                                nc.tensor.matmul(psv[i], ones_sb, bv_sb,
                                                 start=True, stop=False)
                            for kt in range(8):
                                xt = xpool.tile([128, 1024], DT, tag="x",
                                                name=f"xv_{half}_{kt}")
                                nc.sync.dma_start(
                                    xt, xv_ap[kt * 128:(kt + 1) * 128,
                                              half * 1024:(half + 1) * 1024])
                                for i in range(8):
                                    nc.tensor.matmul(
                                        psv[i],
                                        xt[:, i * 128:(i + 1) * 128],
                                        wv_sb[:, kt, :],
                                        start=False, stop=(kt == 7))
                            for i in range(8):
                                tt = half * 8 + i
                                nc.vector.tensor_copy(v_sb[:, tt, :], psv[i])
                with tc.tile_pool(name="pt", bufs=8) as ptpool, \
                     tc.tile_pool(name="sm", bufs=2) as smpool, \
                     tc.tile_pool(name="ysb", bufs=3) as ypool, \
                     tc.tile_pool(name="aux", bufs=1,
                                  space="PSUM") as aux, \
                     tc.tile_pool(name="scps", bufs=2,
                                  space="PSUM") as scps, \
                     tc.tile_pool(name="pvps", bufs=2,
                                  space="PSUM") as pvps:
                    yproj_pool.append(ypool)
                    auxps.append(aux)
                    for p in range(2):
                        emit_attention_pair(p, scps, pvps, ptpool, smpool)

    nc.compile()
    return nc


def _get_program(mode):
    if mode not in _PROGRAMS:
        _PROGRAMS[mode] = _build(mode)
    return _PROGRAMS[mode]


def kernel(q, k, v, mask, Wq, bq, Wk, bk, Wv, bv, Wo, bo):
    global LAST_RESULT
    from concourse.bass_utils import run_bass_kernel_spmd

    mode = MODE
    nc = _get_program(mode)

    if mode == "bf16":
        import ml_dtypes
        cdt = ml_dtypes.bfloat16
    else:
        cdt = np.float32

    def prep(a):
        return np.ascontiguousarray(np.asarray(a).astype(cdt))

    q = np.asarray(q); k = np.asarray(k); v = np.asarray(v)
    Wq = np.asarray(Wq); Wk = np.asarray(Wk); Wv = np.asarray(Wv)
    Wo = np.asarray(Wo)
    bq = np.asarray(bq); bk = np.asarray(bk); bv = np.asarray(bv)
    bo = np.asarray(bo)

    xT = [[prep(q[b].T), prep(k[b].T), prep(v[b].T)] for b in range(B)]

    in_maps = []
    for core in range(NCORES):
        b, g = core // 4, core % 4
        r0 = g * FPC

        def wqk_layout(W):
            # lhsT tiles: [part p, ktile, m] = W.T[kt*128+p, m]
            A = W[r0:r0 + FPC, :].T.reshape(8, 128, FPC)
            return prep(A.transpose(1, 0, 2))

        WvT = Wv[r0:r0 + FPC, :].T  # (E, 256)
        Wv_aug = np.zeros((E, VW), np.float32)
        bv_aug = np.zeros((1, VW), np.float32)
        for h in range(4):
            Wv_aug[:, 65 * h:65 * h + 64] = WvT[:, 64 * h:64 * h + 64]
            bv_aug[0, 65 * h:65 * h + 64] = bv[r0 + 64 * h:r0 + 64 * h + 64]
            bv_aug[0, 65 * h + 64] = 1.0
        Wo_l = Wo[:, r0:r0 + FPC].T.reshape(2, 128, E).transpose(1, 0, 2)

        in_maps.append({
            "xq": xT[b][0], "xk": xT[b][1], "xv": xT[b][2],
            "wq": wqk_layout(Wq),
            "wk": wqk_layout(Wk),
            "wv": prep(Wv_aug.reshape(8, 128, VW).transpose(1, 0, 2)),
            "wo": prep(Wo_l),
            "bqk": np.stack([bq[r0:r0 + 128], bq[r0 + 128:r0 + FPC],
                             bk[r0:r0 + 128], bk[r0 + 128:r0 + FPC]],
                            axis=1).astype(np.float32),
            "bv": prep(bv_aug),
            "ones": np.ones((1, 128), cdt),
        })

    kwargs = {}
    if TRACE:
        kwargs = {"trace": True, "tmpdir": TRACE_DIR}
    res = run_bass_kernel_spmd(nc, in_maps, list(range(NCORES)), **kwargs)
    LAST_RESULT = res

    y = np.zeros((B, S, E), np.float32)
    for core in range(NCORES):
        y[core // 4] += res.results[core]["y"]
    y += bo.astype(np.float32)
    return y



# revision 2
# speedup vs baseline: 1.3005x; 1.3005x over previous
"""Multi-head attention (B=2, S=2048, E=1024, H=16) on 8 Trainium2 cores.

Sharding: core c -> (batch b = c//4, head-group g = c%4 of 4 heads).
Each core computes Q/K/V projections for its 4 heads (256 features),
full attention for those heads, and a partial output projection
(256 rows of Wo). Host sums the 4 partials per batch element and adds bo.

Schedule (v2): built for engine overlap.
  - x DMAs all ride ONE queue (sync) in priority order xk, xv, xq so each
    tensor streams at full HBM bandwidth and the consumers below can track
    arrival; weights ride the gpsimd queue.
  - Lead-in: K-proj(pair0) tracks xk feature tiles (f-inner, 4 psum
    accumulators), V-proj tracks xv (two 8-bank passes), Q-proj(pair0)
    tracks xq. All on one rotating 8-slot psum tag.
  - Attention is software-pipelined one key-tile deep: PE emits
    scores(kt), then PV(kt-1), so the exp(kt) on the Scalar engine
    overlaps PE work instead of stalling it (which also lets the PE
    p-state ramp to 2.4 GHz).
  - pair-1 K/Q projections run as PE filler chunks inside pair-0's
    attention (single rotating psum bank); the output projection
    interleaves into pair-1's attention the same way.

On-chip layouts (per core):
  qt/kt: (128 feat-part, pair, 2048 tok)  transposed proj outputs; the
         128 partitions hold two heads (64+64) per pair index.
  v:     (128 tok-part, 16 tok-tiles, 4*65): per head 64 dims plus a
         "ones" column produced by an augmented V projection (extra
         output feature with zero weights and bias 1.0); P @ V_aug then
         also yields the softmax denominator row for free.
  scores are computed transposed (key-pos on partitions, query on free)
  so exp runs on ACT along the free dim and P tiles feed P@V directly as
  the moving operand; no transposes anywhere in the pipeline.

All weight matrices are re-laid-out on the host so every DMA is a
contiguous per-partition run.
"""

import numpy as np

B, S, E, H = 2, 2048, 1024, 16
D = 64
NCORES = 8
FPC = 256  # features (head dims) per core = 4 heads
VW = 4 * 65  # V-projection output width incl. ones columns

MODE = "bf16"

_PROGRAMS = {}
LAST_RESULT = None
TRACE = False
TRACE_DIR = None


def _build(mode):
    import concourse.tile as tile
    from concourse import bacc, mybir

    f32 = mybir.dt.float32
    DT = mybir.dt.bfloat16
    NW = 512
    NNB = S // NW  # 4 query blocks per pair

    nc = bacc.Bacc("TRN2", target_bir_lowering=False, debug=False,
                   num_devices=NCORES)

    xq_ap = nc.dram_tensor("xq", [E, S], DT, kind="ExternalInput").ap()
    xk_ap = nc.dram_tensor("xk", [E, S], DT, kind="ExternalInput").ap()
    xv_ap = nc.dram_tensor("xv", [E, S], DT, kind="ExternalInput").ap()
    wq_ap = nc.dram_tensor("wq", [128, 8, FPC], DT, kind="ExternalInput").ap()
    wk_ap = nc.dram_tensor("wk", [128, 8, FPC], DT, kind="ExternalInput").ap()
    wv_ap = nc.dram_tensor("wv", [128, 8, VW], DT, kind="ExternalInput").ap()
    wo_ap = nc.dram_tensor("wo", [128, 2, E], DT, kind="ExternalInput").ap()
    bqk_ap = nc.dram_tensor("bqk", [128, 4], f32, kind="ExternalInput").ap()
    bv_ap = nc.dram_tensor("bv", [1, VW], DT, kind="ExternalInput").ap()
    ones_ap = nc.dram_tensor("ones", [1, 128], DT, kind="ExternalInput").ap()
    y_ap = nc.dram_tensor("y", [S, E], f32, kind="ExternalOutput").ap()

    Exp = mybir.ActivationFunctionType.Exp

    with tile.TileContext(nc) as tc:
        with tc.tile_pool(name="persist", bufs=1) as persist:
            wq_sb = persist.tile([128, 8, FPC], DT, name="wq_sb")
            wk_sb = persist.tile([128, 8, FPC], DT, name="wk_sb")
            wv_sb = persist.tile([128, 8, VW], DT, name="wv_sb")
            wo_sb = persist.tile([128, 2, E], DT, name="wo_sb")
            bqk_sb = persist.tile([128, 4], f32, name="bqk_sb")
            bv_sb = persist.tile([1, VW], DT, name="bv_sb")
            ones_sb = persist.tile([1, 128], DT, name="ones_sb")
            qt_sb = persist.tile([128, 2, S], DT, name="qt_sb")
            kt_sb = persist.tile([128, 2, S], DT, name="kt_sb")
            v_sb = persist.tile([128, 16, VW], DT, name="v_sb")
            at_sb = persist.tile([128, 2, S], DT, name="at_sb")
            xq_res = persist.tile([128, 8, S], DT, name="xq_res")
            xk_res = persist.tile([128, 8, S], DT, name="xk_res")
            xv_res = persist.tile([128, 8, S], DT, name="xv_res")
            ones32 = ones_sb[:, 0:64]

            # weights/consts on the gpsimd DGE queue, k first (needed first)
            nc.gpsimd.dma_start(wk_sb, wk_ap)
            nc.gpsimd.dma_start(bqk_sb, bqk_ap)
            nc.gpsimd.dma_start(wv_sb, wv_ap)
            nc.gpsimd.dma_start(bv_sb, bv_ap)
            nc.gpsimd.dma_start(ones_sb, ones_ap)
            nc.gpsimd.dma_start(wq_sb, wq_ap)
            nc.gpsimd.dma_start(wo_sb, wo_ap)
            # all x on ONE queue so later tensors never steal bandwidth
            # from earlier ones: priority xk > xv > xq.
            for f in range(8):
                nc.sync.dma_start(xk_res[:, f, :],
                                  xk_ap[f * 128:(f + 1) * 128, :])
            for f in range(8):
                nc.sync.dma_start(xv_res[:, f, :],
                                  xv_ap[f * 128:(f + 1) * 128, :])
            for f in range(8):
                nc.sync.dma_start(xq_res[:, f, :],
                                  xq_ap[f * 128:(f + 1) * 128, :])

            def emit_qk_proj(pool, p, w_sb, x_res, out_sb, bcol, tag, bufs):
                """f-inner projection for one pair: 4 psum accumulators
                track the x feature tiles as they arrive."""
                pj = [pool.tile([128, NW], f32, tag=tag, bufs=bufs,
                                name=f"pj_{bcol}_{p}_{nb}")
                      for nb in range(NNB)]
                for f in range(8):
                    for nb in range(NNB):
                        nc.tensor.matmul(
                            pj[nb],
                            w_sb[:, f, p * 128:(p + 1) * 128],
                            x_res[:, f, nb * NW:(nb + 1) * NW],
                            start=(f == 0), stop=(f == 7))
                for nb in range(NNB):
                    nc.vector.tensor_scalar_add(
                        out_sb[:, p, nb * NW:(nb + 1) * NW], pj[nb],
                        bqk_sb[:, bcol + p:bcol + p + 1])

            def emit_v_half(pool, half):
                """V projection for 8 token tiles, f-inner (tracks xv)."""
                psv = [pool.tile([128, VW], f32, tag="proj", bufs=8,
                                 name=f"pjv_{half}_{i}")
                       for i in range(8)]
                for i in range(8):
                    nc.tensor.matmul(psv[i], ones_sb, bv_sb,
                                     start=True, stop=False)
                for f in range(8):
                    for i in range(8):
                        tt = half * 8 + i
                        nc.tensor.matmul(
                            psv[i],
                            xv_res[:, f, tt * 128:(tt + 1) * 128],
                            wv_sb[:, f, :],
                            start=False, stop=(f == 7))
                for i in range(8):
                    nc.vector.tensor_copy(v_sb[:, half * 8 + i, :], psv[i])

            # ---- lead-in: K-proj p0, V-proj, Q-proj p0 ----
            with tc.tile_pool(name="lead", bufs=1, space="PSUM") as lead:
                emit_qk_proj(lead, 0, wk_sb, xk_res, kt_sb, 2, "proj", 8)
                emit_v_half(lead, 0)
                emit_v_half(lead, 1)
                emit_qk_proj(lead, 0, wq_sb, xq_res, qt_sb, 0, "proj", 8)

            # ---- attention (+ interleaved fillers) ----
            with tc.tile_pool(name="pt", bufs=8) as ptpool, \
                 tc.tile_pool(name="sm", bufs=2) as smpool, \
                 tc.tile_pool(name="ysb", bufs=2) as ypool, \
                 tc.tile_pool(name="scps", bufs=2, space="PSUM") as scps, \
                 tc.tile_pool(name="pvps", bufs=2, space="PSUM") as pvps, \
                 tc.tile_pool(name="miscps", bufs=1, space="PSUM") as miscps:

                # pair-1 projections, emitted lazily one chunk (= one
                # query/key block) at a time as PE filler during pair 0.
                def p1_proj_chunks():
                    for w_sb, out_sb, bcol in ((wk_sb, kt_sb, 2),
                                               (wq_sb, qt_sb, 0)):
                        for nb in range(NNB):
                            yield (w_sb, out_sb, bcol, nb)

                filler_iter = p1_proj_chunks()

                def emit_filler():
                    ch = next(filler_iter, None)
                    if ch is None:
                        return
                    w_sb, out_sb, bcol, nb = ch
                    pj = miscps.tile([128, NW], f32, tag="misc",
                                     name=f"pj1_{bcol}_{nb}")
                    for f in range(8):
                        nc.tensor.matmul(
                            pj,
                            w_sb[:, f, 128:256],
                            (xk_res if bcol == 2 else xq_res)
                            [:, f, nb * NW:(nb + 1) * NW],
                            start=(f == 0), stop=(f == 7))
                    nc.vector.tensor_scalar_add(
                        out_sb[:, 1, nb * NW:(nb + 1) * NW], pj,
                        bqk_sb[:, bcol + 1:bcol + 2])

                def emit_yproj_block(qb):
                    for mt in range(4 * qb, 4 * qb + 4):
                        yo = ypool.tile([128, E], f32, tag="yo",
                                        name=f"yo_{mt}")
                        for nb in range(2):
                            yp = miscps.tile([128, NW], f32, tag="misc",
                                             name=f"yp_{mt}_{nb}")
                            for p2 in range(2):
                                nc.tensor.matmul(
                                    yp,
                                    at_sb[:, p2, mt * 128:(mt + 1) * 128],
                                    wo_sb[:, p2, nb * NW:(nb + 1) * NW],
                                    start=(p2 == 0), stop=(p2 == 1))
                            nc.vector.tensor_copy(
                                yo[:, nb * NW:(nb + 1) * NW], yp)
                        eng = nc.gpsimd if mt % 2 else nc.scalar
                        eng.dma_start(y_ap[mt * 128:(mt + 1) * 128, :], yo)

                for p in range(2):
                    for qb in range(NNB):
                        qsl = slice(qb * NW, (qb + 1) * NW)
                        pvt = [pvps.tile([65, NW], f32, tag="pv",
                                         name=f"pv_{p}_{qb}_{hh}")
                               for hh in range(2)]
                        prev = None

                        def emit_pv(kt, ptt):
                            for hh in range(2):
                                h = 2 * p + hh
                                nc.tensor.matmul(
                                    pvt[hh],
                                    v_sb[:, kt, 65 * h:65 * h + 65],
                                    ptt[:, NW * hh:NW * hh + NW],
                                    start=(kt == 0), stop=(kt == 15))

                        for kt in range(16):
                            s_ = scps.tile([128, 2 * NW], f32, tag="sc",
                                           name=f"sc_{p}_{qb}_{kt}")
                            for hh in range(2):
                                nc.tensor.matmul(
                                    s_[:, NW * hh:NW * hh + NW],
                                    kt_sb[64 * hh:64 * hh + 64, p,
                                          kt * 128:(kt + 1) * 128],
                                    qt_sb[64 * hh:64 * hh + 64, p, qsl],
                                    start=True, stop=True)
                            ptt = ptpool.tile([128, 2 * NW], DT, tag="pt",
                                              name=f"pt_{p}_{qb}_{kt}")
                            nc.scalar.activation(ptt, s_, Exp, scale=0.125)
                            if prev is not None:
                                emit_pv(*prev)
                            prev = (kt, ptt)
                            # one pair-1 projection chunk every 8 kt units
                            if p == 0 and kt in (3, 11):
                                emit_filler()
                        emit_pv(*prev)

                        for hh in range(2):
                            denr = smpool.tile([1, NW], DT, tag="denr",
                                               name=f"dn_{p}_{qb}_{hh}")
                            nc.vector.tensor_copy(denr, pvt[hh][64:65, :])
                            rb = pvps.tile([64, NW], f32, tag="rb", bufs=1,
                                           name=f"rb_{p}_{qb}_{hh}")
                            nc.tensor.matmul(rb, ones32, denr,
                                             start=True, stop=True)
                            rbs = smpool.tile([64, NW], f32, tag="rbs",
                                              name=f"rbs_{p}_{qb}_{hh}")
                            nc.vector.reciprocal_approx_fast(rbs, rb)
                            nc.vector.tensor_mul(
                                at_sb[64 * hh:64 * hh + 64, p, qsl],
                                pvt[hh][0:64, :], rbs)
                        if p == 1:
                            emit_yproj_block(qb)

    nc.compile()
    return nc


def _get_program(mode):
    if mode not in _PROGRAMS:
        _PROGRAMS[mode] = _build(mode)
    return _PROGRAMS[mode]


def kernel(q, k, v, mask, Wq, bq, Wk, bk, Wv, bv, Wo, bo):
    global LAST_RESULT
    from concourse.bass_utils import run_bass_kernel_spmd

    mode = MODE
    nc = _get_program(mode)

    import ml_dtypes
    cdt = ml_dtypes.bfloat16

    def prep(a):
        return np.ascontiguousarray(np.asarray(a).astype(cdt))

    q = np.asarray(q); k = np.asarray(k); v = np.asarray(v)
    Wq = np.asarray(Wq); Wk = np.asarray(Wk); Wv = np.asarray(Wv)
    Wo = np.asarray(Wo)
    bq = np.asarray(bq); bk = np.asarray(bk); bv = np.asarray(bv)
    bo = np.asarray(bo)

    xT = [[prep(q[b].T), prep(k[b].T), prep(v[b].T)] for b in range(B)]

    in_maps = []
    for core in range(NCORES):
        b, g = core // 4, core % 4
        r0 = g * FPC

        def wqk_layout(W):
            # lhsT tiles: [part p, ktile, m] = W.T[kt*128+p, m]
            A = W[r0:r0 + FPC, :].T.reshape(8, 128, FPC)
            return prep(A.transpose(1, 0, 2))

        WvT = Wv[r0:r0 + FPC, :].T  # (E, 256)
        Wv_aug = np.zeros((E, VW), np.float32)
        bv_aug = np.zeros((1, VW), np.float32)
        for h in range(4):
            Wv_aug[:, 65 * h:65 * h + 64] = WvT[:, 64 * h:64 * h + 64]
            bv_aug[0, 65 * h:65 * h + 64] = bv[r0 + 64 * h:r0 + 64 * h + 64]
            bv_aug[0, 65 * h + 64] = 1.0
        Wo_l = Wo[:, r0:r0 + FPC].T.reshape(2, 128, E).transpose(1, 0, 2)

        in_maps.append({
            "xq": xT[b][0], "xk": xT[b][1], "xv": xT[b][2],
            "wq": wqk_layout(Wq),
            "wk": wqk_layout(Wk),
            "wv": prep(Wv_aug.reshape(8, 128, VW).transpose(1, 0, 2)),
            "wo": prep(Wo_l),
            "bqk": np.stack([bq[r0:r0 + 128], bq[r0 + 128:r0 + FPC],
                             bk[r0:r0 + 128], bk[r0 + 128:r0 + FPC]],
                            axis=1).astype(np.float32),
            "bv": prep(bv_aug),
            "ones": np.ones((1, 128), cdt),
        })

    kwargs = {}
    if TRACE:
        kwargs = {"trace": True, "tmpdir": TRACE_DIR}
    res = run_bass_kernel_spmd(nc, in_maps, list(range(NCORES)), **kwargs)
    LAST_RESULT = res

    y = np.zeros((B, S, E), np.float32)
    for core in range(NCORES):
        y[core // 4] += res.results[core]["y"]
    y += bo.astype(np.float32)
    return y
